# revision 1
# baseline (speedup 1.0000x reference)
"""Trainium2 Bass kernel for nn_FComb_79319456023150 (dense_cnn).

Per-pixel MLP over a 96^3 volume: four 1x1x1 convs (38->32->32->32->1 channels
with relu between). z is batch-constant, so w1[:, 32:38] @ z folds into the
layer-1 bias and every layer becomes a K=32 channel GEMM.

Sharding: spatial (outermost X axis) across 8 cores, 110592 pixels each.
Weights/biases replicated.

Device layout per core: the host restripes each shard to [128, 27648] = 4
pixel-blocks x 32 channels on partitions, pixels on the free dim. Each layer
is computed with a BLOCK-DIAGONAL [128, 128] weight (4 copies of W^T on the
diagonal), so one full-array float32r matmul per 512-pixel chunk applies the
32x32 GEMM to all 4 pixel blocks at once (1 col/cycle). The final layer
(wl: 1x32) uses one sparse [128, 128] weight per chunk whose outputs land on
contiguous partitions 4c+m; accumulating the chunk matmuls into one PSUM
bank packs a whole super-chunk's output into rows 0..OROWS-1 for a single
cheap evacuation op and batched, affine output DMAs.

Relu+bias rides the mandatory PSUM->SBUF crossing as ONE whole-crossing op
per layer, alternating between ScalarE (activation Relu w/ bias) and VectorE
(fused tensor_scalar add+max) by (s+layer) parity — these two engines are
the throughput bound (fp32-from-PSUM is 1x on both), and whole ops amortize
their fixed per-op cost best while keeping the two engines' dependency
chains decoupled. FOUR independent super-chunk pipelines (s%4), each owning
one 2-bank PSUM slot (the L4 accumulator reuses the slot after relu3 drains
it), keep both engines ~75% busy. Input DMAs ramp up (4 single-sc loads,
then 3-sc batches) so the pipeline starts early; each HWDGE dma_start costs
~0.65us of issuing-sequencer time, hence the batching.
"""

import sys

import numpy as np

if "/opt/trn_rl_repo" not in sys.path:
    sys.path.insert(0, "/opt/trn_rl_repo")

C = 32          # channels per layer
P = 128         # SBUF/PSUM partitions
RG = 4          # pixel blocks stacked on the partition dim (128/32)
NCHUNK = 2      # 512-wide chunks per super-chunk (PSUM big tile = 2 banks)
CH = 512        # chunk width (one PSUM bank of fp32)
SCW = NCHUNK * CH                    # 1536 free-dim columns per super-chunk
VOL = 96 * 96 * 96                   # full volume
NCORES = 8
NPIX = VOL // NCORES                 # 110592 pixels per core
FREE = NPIX // RG                    # 27648 free-dim columns per core
NSC = FREE // SCW                    # 18 super-chunks per core
OROWS = RG * NCHUNK                  # 12 packed output rows per super-chunk
assert FREE % SCW == 0



def _pick_group(nsc, target):
    for g in range(min(target, nsc), 0, -1):
        if nsc % g == 0:
            return g
    return 1


def _build_nc(npix=NPIX, use_f32r=True, stagger=False, mirror=False):
    import concourse.mybir as mybir
    from concourse import bacc
    from concourse.tile import TileContext
    from concourse.tile_rust import add_dep_helper

    f32 = mybir.dt.float32
    f32r = mybir.dt.float32r if use_f32r else mybir.dt.float32
    Alu = mybir.AluOpType
    Act = mybir.ActivationFunctionType

    free = npix // RG
    nsc = free // SCW
    assert free % SCW == 0 and nsc >= 1
    gin = _pick_group(nsc, 3)        # super-chunks per input DMA
    gout = _pick_group(nsc, 27)       # super-chunks per output tile/DMA group

    nc = bacc.Bacc()
    fm = nc.dram_tensor("fm", [P, free], f32r, kind="ExternalInput")
    wst = nc.dram_tensor("wst", [P, (3 + NCHUNK) * P], f32r, kind="ExternalInput")
    bias = nc.dram_tensor("bias", [P, 4], f32, kind="ExternalInput")
    out = nc.dram_tensor("out", [npix], f32, kind="ExternalOutput")

    # out[m*free + s*SCW + c*CH + n] viewed for batched affine stores
    out_r = out.rearrange(
        "(m go g c n) -> m go g c n", m=RG, go=nsc // gout, g=gout, c=NCHUNK, n=CH
    )

    with TileContext(nc) as tc:
        with (
            tc.tile_pool(name="const", bufs=1) as constp,
            tc.tile_pool(name="data", bufs=4) as datap,
            tc.tile_pool(name="acts", bufs=4) as actp,
            tc.tile_pool(name="outs", bufs=2) as outsp,
            tc.tile_pool(name="psb", bufs=1, space="PSUM") as psb,
        ):
            wtile = constp.tile([P, (3 + NCHUNK) * P], f32r)
            nc.sync.dma_start(wtile, wst[:, :])
            btile = constp.tile([P, 4], f32)
            nc.sync.dma_start(btile, bias[:, :])

            # Input DMA groups: first few single-sc loads so the pipeline
            # starts after ~0.5 MB instead of a full multi-sc transfer, then
            # steady-state groups of `gin` super-chunks.
            groups = [1] * min(4, nsc)
            while sum(groups) < nsc:
                groups.append(min(gin, nsc - sum(groups)))
            group_of = []
            for gidx, g in enumerate(groups):
                group_of += [(gidx, len(group_of), g)] * g
            group_starts = {}
            for s_, (gidx, gbase, g) in enumerate(group_of):
                group_starts.setdefault(gidx, (s_, g))

            xbig = None
            xbase = 0
            ob = None
            sc0_gate = None       # sc0's relu2 op, used to stagger stream B
            for s in range(nsc):
                gidx, gbase, gwidth = group_of[s]
                if s == gbase:
                    xbig = datap.tile([P, gwidth * SCW], f32r, tag="x")
                    xbase = gbase
                    nc.sync.dma_start(
                        xbig, fm[:, gbase * SCW:(gbase + gwidth) * SCW]
                    )
                si = s - xbase
                h = xbig[:, si * SCW:(si + 1) * SCW]

                # Four independent sc streams (s%4), each owning one
                # 2-bank PSUM slot: within a stream, relu(l) must complete
                # before mm(l+1) anyway, so one slot costs nothing, while
                # the streams interleave freely on every engine.
                for layer in range(3):
                    ps = psb.tile([P, SCW], f32, tag=f"ps{s % 4}")
                    wsl = wtile[:, layer * P:(layer + 1) * P]
                    for cc in range(NCHUNK):
                        mm = nc.tensor.matmul(
                            ps[:, cc * CH:(cc + 1) * CH],
                            wsl,
                            h[:, cc * CH:(cc + 1) * CH],
                            start=True,
                            stop=True,
                        )
                        NAME_INFO[mm.ins.name] = (s, f"mm{layer}.{cc}")
                        if stagger and s == 1 and layer == 0 and cc == 0 \
                                and sc0_gate is not None:
                            add_dep_helper(sc0_gate, mm.ins,
                                           reason="stagger stream B")
                    hn = actp.tile([P, SCW], f32r, tag=f"h{layer}")
                    bcol = btile[:, layer:layer + 1]
                    # Whole-crossing relu on one engine, alternating by
                    # (s + layer): each crossing is a single large op (best
                    # per-op amortization) and the two engines' dependency
                    # chains stay decoupled across layers.
                    if (s + layer) % 2 == 0:
                        xop = nc.scalar.activation(
                            hn[:, :], ps[:, :], Act.Relu,
                            bias=bcol, scale=1.0,
                        )
                        NAME_INFO[xop.ins.name] = (s, f"reluA{layer}")
                    else:
                        xop = nc.vector.tensor_scalar(
                            hn[:, :], ps[:, :],
                            bcol, 0.0, Alu.add, Alu.max,
                        )
                        NAME_INFO[xop.ins.name] = (s, f"reluD{layer}")
                    if s == 0 and layer == 1:
                        sc0_gate = xop.ins
                    h = hn

                # Layer 4: chunk c's [128,128] weight has wl only in columns
                # 4c+m (m<4); accumulating the 3 chunk matmuls into one bank
                # leaves out[4c+m, n] = wl @ (block m of chunk c) on the
                # contiguous partitions 0..11.
                go, so = divmod(s, gout)
                if so == 0:
                    ob = outsp.tile([OROWS, gout * CH], f32, tag="ob")
                ps4 = psb.tile([P, CH], f32, tag=f"ps{s % 4}")
                for cc in range(NCHUNK):
                    mm4 = nc.tensor.matmul(
                        ps4[:, :],
                        wtile[:, (3 + cc) * P:(4 + cc) * P],
                        h[:, cc * CH:(cc + 1) * CH],
                        start=(cc == 0),
                        stop=(cc == NCHUNK - 1),
                    )
                    NAME_INFO[mm4.ins.name] = (s, f"mm4.{cc}")
                blcol = btile[:OROWS, 3:4]
                if s % 2 == 0 and s % 8 != 0:
                    fin = nc.vector.tensor_scalar(
                        ob[:, so * CH:(so + 1) * CH], ps4[:OROWS, :],
                        blcol, None, Alu.add,
                    )
                else:
                    fin = nc.scalar.activation(
                        ob[:, so * CH:(so + 1) * CH], ps4[:OROWS, :],
                        Act.Identity, bias=blcol, scale=1.0,
                    )
                NAME_INFO[fin.ins.name] = (s, "final")
                # Store in two waves: the first ~2/3 of the output ships
                # while compute continues, so only the last third's DMA sits
                # in the drain tail.
                if gout == nsc:
                    wsplit = max(1, 8 * nsc // 9)
                    waves = {wsplit - 1: (0, wsplit), nsc - 1: (wsplit, nsc)}
                    if s in waves:
                        a, b = waves[s]
                        for cc in range(NCHUNK):
                            nc.sync.dma_start(
                                out_r[:, 0, a:b, cc, :],
                                ob[RG * cc:RG * cc + RG, a * CH:b * CH].rearrange(
                                    "m (g n) -> m g n", n=CH
                                ),
                            )
                elif so == gout - 1:
                    for cc in range(NCHUNK):
                        nc.sync.dma_start(
                            out_r[:, go, :, cc, :],
                            ob[RG * cc:RG * cc + RG, :].rearrange(
                                "m (g n) -> m g n", n=CH
                            ),
                        )

    # Walrus codegen cannot reliably attach semaphore waits to self-loading
    # matmuls; hoist every matmul's waits onto a PE nop inserted just before
    # it (sequencer-side wait, same semantics).
    for blk in nc.main_func.blocks:
        insts = blk.instructions
        idx = 0
        while idx < len(insts):
            inst = insts[idx]
            if isinstance(inst, mybir.InstMatmult):
                si = inst.sync_info
                if si is not None and len(si.on_wait) > 0:
                    nop = mybir.InstNoOp(
                        name=nc.get_next_instruction_name(), ins=[], outs=[]
                    )
                    nop.engine = inst.engine
                    nop.bass_nofuse = True
                    nop.sync_info = mybir.SyncInfo(on_wait=si.on_wait, on_update=[])
                    si.on_wait = []
                    nc.register_instruction(nop)
                    insts.insert(idx, nop)
                    idx += 1
            idx += 1

    for blk in nc.main_func.blocks:
        for inst in blk.instructions:
            if isinstance(inst, mybir.InstMatmult):
                si = inst.sync_info
                assert si is None or len(si.on_wait) == 0, inst.name

    nc.compile()
    return nc


def _blockdiag4(wT):
    """[32, 32] -> [128, 128] block-diagonal with 4 copies."""
    out = np.zeros((P, P), dtype=np.float32)
    for b in range(RG):
        out[32 * b:32 * b + 32, 32 * b:32 * b + 32] = wT
    return out


def _prep_host_inputs(z, w1, b1, w2, b2, w3, b3, wl, bl):
    """Fold z into the layer-1 bias and build the device weight layouts."""
    f32 = np.float32
    b1e = (b1 + w1[:, C:] @ z[0]).astype(f32)          # [32]

    w4 = np.zeros((P, NCHUNK * P), dtype=f32)
    for cc in range(NCHUNK):
        for m in range(RG):
            w4[32 * m:32 * m + 32, cc * P + RG * cc + m] = wl[0, :]

    wst = np.concatenate(
        [
            _blockdiag4(w1[:, :C].T),
            _blockdiag4(w2.T),
            _blockdiag4(w3.T),
            w4,
        ],
        axis=1,
    ).astype(f32)                                       # [128, 768]

    bias = np.zeros((P, 4), dtype=f32)
    bias[:, 0] = np.tile(b1e, RG)
    bias[:, 1] = np.tile(b2.astype(f32), RG)
    bias[:, 2] = np.tile(b3.astype(f32), RG)
    bias[:, 3] = f32(bl[0])
    return wst, bias


def _restripe(shard):
    """[32, npix] channel-major shard -> [128, npix/4] (block, channel) rows."""
    npix = shard.shape[1]
    return np.ascontiguousarray(
        shard.reshape(C, RG, npix // RG).transpose(1, 0, 2).reshape(P, npix // RG)
    )


_NC_CACHE = {}
NAME_INFO = {}   # instruction name -> (sc, stage) for profiling


def _run(feature_map, z, w1, b1, w2, b2, w3, b3, wl, bl, **spmd_kwargs):
    from concourse.bass_utils import run_bass_kernel_spmd

    feature_map = np.asarray(feature_map, dtype=np.float32)
    z = np.asarray(z, dtype=np.float32)
    w1, b1 = np.asarray(w1, np.float32), np.asarray(b1, np.float32)
    w2, b2 = np.asarray(w2, np.float32), np.asarray(b2, np.float32)
    w3, b3 = np.asarray(w3, np.float32), np.asarray(b3, np.float32)
    wl, bl = np.asarray(wl, np.float32), np.asarray(bl, np.float32)

    wst, bias = _prep_host_inputs(z, w1, b1, w2, b2, w3, b3, wl, bl)

    fm_flat = feature_map.reshape(C, VOL)
    in_maps = []
    for k in range(NCORES):
        shard = _restripe(fm_flat[:, k * NPIX:(k + 1) * NPIX])
        in_maps.append({"fm": shard, "wst": wst, "bias": bias})

    if "nc" not in _NC_CACHE:
        _NC_CACHE["nc"] = _build_nc()
    nc = _NC_CACHE["nc"]

    res = run_bass_kernel_spmd(nc, in_maps, core_ids=list(range(NCORES)), **spmd_kwargs)
    out = np.empty((VOL,), dtype=np.float32)
    for k in range(NCORES):
        out[k * NPIX:(k + 1) * NPIX] = res.results[k]["out"]
    return out.reshape(1, 1, 96, 96, 96), res


def kernel(feature_map, z, w1, b1, w2, b2, w3, b3, wl, bl):
    out, _ = _run(feature_map, z, w1, b1, w2, b2, w3, b3, wl, bl)
    return out



# revision 5
# speedup vs baseline: 1.0973x; 1.0973x over previous
"""Trainium2 Bass kernel for nn_FComb_79319456023150 (dense_cnn).

Per-pixel MLP over a 96^3 volume: four 1x1x1 convs (38->32->32->32->1 channels
with relu between). z is batch-constant, so w1[:, 32:38] @ z folds into the
layer-1 bias and every layer becomes a K=32 channel GEMM.

Sharding: spatial (outermost X axis) across 8 cores, 110592 pixels each.
Weights/biases replicated.

Device layout per core: the host restripes each shard to [128, 27648] =
4 pixel-blocks x 32 channels on partitions, pixels on the free dim, cast to
bf16 (halves HBM traffic; fp32 PSUM accumulation keeps rel-err ~5e-3).
Layers 1-3 use a block-diagonal [128, 128] bf16 weight (4 copies of W^T on
the diagonal), so one matmul applies the 32x32 GEMM to 4 pixel blocks at
once. Work is split into 512-column slabs cycling over SEVEN single-bank
PSUM regions; the eighth bank accumulates layer 4: chunk c's sparse [128,128]
weight lands wl @ h3 on psum rows 4c+m, so one wide [rows, 512] crossing
evacuates an entire accumulation group and batched affine DMAs ship it.

Relu rides the mandatory PSUM->SBUF crossing as one whole-crossing op per
slab-layer, alternating ScalarE (activation Relu w/ bias) and VectorE (fused
tensor_scalar add+max) via a planner that equalizes modeled engine time.
Seven regions keep ~6 crossings in flight so both crossing engines stay
~85% busy, which is the throughput bound for this kernel.
"""

import sys

import numpy as np

if "/opt/trn_rl_repo" not in sys.path:
    sys.path.insert(0, "/opt/trn_rl_repo")

import ml_dtypes

C = 32
P = 128
RG = 4
CH = 512
VOL = 96 * 96 * 96
NCORES = 8
NPIX = VOL // NCORES       # 110592
FREE = NPIX // RG          # 27648
NCHUNKS = FREE // CH       # 54

BF16_NP = ml_dtypes.bfloat16

CFG = dict(
    l4_groups=(18, 18, 18),
    dma_targets=(512, 1024, 2048),
    dma_steady=3072,
    slab_pattern=(512,) * 7,
    act_share=0.51,
)

NAME_INFO = {}


def _slabs():
    pat = CFG["slab_pattern"]
    out = []
    col, i = 0, 0
    while col < FREE:
        r = i % len(pat)
        w = min(pat[r], FREE - col)
        out.append((col, w, r))
        col += w
        i += 1
    return out


def _build_nc(npix=NPIX):
    import concourse.mybir as mybir
    from concourse import bacc
    from concourse.tile import TileContext

    cfg = CFG
    l4_groups = cfg["l4_groups"]
    assert sum(l4_groups) == NCHUNKS and max(l4_groups) * RG <= P

    f32 = mybir.dt.float32
    bf16 = mybir.dt.bfloat16
    Alu = mybir.AluOpType
    Act = mybir.ActivationFunctionType

    free = npix // RG
    assert free == FREE

    nacc_max = max(l4_groups)

    nc = bacc.Bacc()
    fm = nc.dram_tensor("fm", [P, free], bf16, kind="ExternalInput")
    w13 = nc.dram_tensor("w13", [P, 3 * P], bf16, kind="ExternalInput")
    w4s = nc.dram_tensor("w4s", [P, nacc_max * P], bf16, kind="ExternalInput")
    bias = nc.dram_tensor("bias", [P, 4], f32, kind="ExternalInput")
    out = nc.dram_tensor("out", [npix], f32, kind="ExternalOutput")

    out_r = out.rearrange("(m k n) -> m k n", m=RG, k=NCHUNKS, n=CH)

    slabs = _slabs()

    # Input DMA groups: consecutive slabs batched to ramped byte targets.
    dma_groups = []
    targets = list(cfg["dma_targets"])
    cur, cw, ti = [], 0, 0
    for idx, (c0, w, r) in enumerate(slabs):
        cur.append(idx)
        cw += w
        tgt = targets[ti] if ti < len(targets) else cfg["dma_steady"]
        if cw >= tgt:
            dma_groups.append(cur)
            cur, cw = [], 0
            ti += 1
    if cur:
        dma_groups.append(cur)
    group_of_slab = {}
    for gi, g in enumerate(dma_groups):
        for idx in g:
            group_of_slab[idx] = gi

    l4_of_chunk = {}
    k = 0
    for g, n in enumerate(l4_groups):
        for c in range(n):
            l4_of_chunk[k] = (g, c, n)
            k += 1

    # crossing engine planner: stream-local alternation, seeded to equalize
    # modeled engine time (ACT is faster per op; DVE runs at 0.96 GHz)
    plan_t = {"act": 0.0, "dve": 0.0}

    def op_cost(eng, fr):
        return (fr + 222) / 1.2 if eng == "act" else (fr + 120) / 0.96

    def pick_seed(fr):
        best, arg = None, None
        for first in ("act", "dve"):
            seq = [first, ("dve" if first == "act" else "act"), first]
            t = dict(plan_t)
            for e in seq:
                t[e] += op_cost(e, fr)
            m = max(t["act"] / cfg["act_share"],
                    t["dve"] / (1 - cfg["act_share"]))
            if best is None or m < best:
                best, arg = m, seq
        for e in arg:
            plan_t[e] += op_cost(e, fr)
        return arg

    with TileContext(nc) as tc:
        with (
            tc.tile_pool(name="const", bufs=1) as constp,
            tc.tile_pool(name="data", bufs=4) as datap,
            tc.tile_pool(name="acts", bufs=4) as actp,
            tc.tile_pool(name="outs", bufs=2) as outsp,
            tc.tile_pool(name="psb", bufs=1, space="PSUM") as psb,
        ):
            w13t = constp.tile([P, 3 * P], bf16)
            btile = constp.tile([P, 4], f32)
            w4t = constp.tile([P, nacc_max * P], bf16)

            # dummy relu at t~0 pulls the ACT table load off the ramp
            tiny = constp.tile([1, 2], f32)
            tiny2 = constp.tile([1, 2], f32)
            nc.vector.memset(tiny[:, :], 0.0)
            pre = nc.scalar.activation(
                tiny2[:, :], tiny[:, :], Act.Relu, bias=0.0, scale=1.0
            )
            NAME_INFO[pre.ins.name] = (-1, "preload")

            w4_half = nacc_max * P // 2
            w4_loads = [(0, w4_half), (w4_half, nacc_max * P)]

            xtiles = {}
            ob = None
            ps4 = None
            fired_const = False
            slab_h = {}
            slab_engs = {}

            def emit_stage(s, stage):
                nonlocal ob, ps4, fired_const
                c0, w, r = slabs[s]
                nch = w // CH
                if stage == 0:
                    gi = group_of_slab[s]
                    if gi not in xtiles:
                        gslabs = dma_groups[gi]
                        ga = slabs[gslabs[0]][0]
                        gb = slabs[gslabs[-1]][0] + slabs[gslabs[-1]][1]
                        xt = datap.tile([P, gb - ga], bf16, tag="x")
                        nc.sync.dma_start(xt, fm[:, ga:gb])
                        xtiles[gi] = (xt, ga)
                        if not fired_const:
                            nc.sync.dma_start(w13t, w13[:, :])
                            nc.sync.dma_start(btile, bias[:, :])
                            fired_const = True
                        elif gi in (1, 2) and w4_loads:
                            a, b = w4_loads.pop(0)
                            nc.sync.dma_start(w4t[:, a:b], w4s[:, a:b])
                    if s >= 3 and w4_loads:
                        a, b = w4_loads.pop(0)
                        nc.sync.dma_start(w4t[:, a:b], w4s[:, a:b])
                    xt, ga = xtiles[group_of_slab[s]]
                    slab_h[s] = xt[:, c0 - ga:c0 - ga + w]
                    slab_engs[s] = pick_seed(w)

                h = slab_h[s]
                if stage < 3:
                    layer = stage
                    ps = psb.tile([P, w], f32, tag=f"ps{r}")
                    wsl = w13t[:, layer * P:(layer + 1) * P]
                    for cc in range(nch):
                        mm = nc.tensor.matmul(
                            ps[:, cc * CH:(cc + 1) * CH],
                            wsl,
                            h[:, cc * CH:(cc + 1) * CH],
                            start=True,
                            stop=True,
                        )
                        NAME_INFO[mm.ins.name] = (s, f"mm{layer}.{cc}")
                    hn = actp.tile([P, w], bf16, tag=f"h{r}{layer}")
                    bcol = btile[:, layer:layer + 1]
                    if slab_engs[s][layer] == "act":
                        xop = nc.scalar.activation(
                            hn[:, :], ps[:, :], Act.Relu, bias=bcol, scale=1.0
                        )
                    else:
                        xop = nc.vector.tensor_scalar(
                            hn[:, :], ps[:, :], bcol, 0.0, Alu.add, Alu.max
                        )
                    NAME_INFO[xop.ins.name] = (s, f"relu{layer}")
                    slab_h[s] = hn
                    return

                # stage 3: layer 4, accumulate into the shared bank
                for cc in range(nch):
                    k = c0 // CH + cc
                    g, c, n_in_g = l4_of_chunk[k]
                    if c == 0:
                        ps4 = psb.tile([P, CH], f32, tag="psL4")
                    mm4 = nc.tensor.matmul(
                        ps4[:, :],
                        w4t[:, c * P:(c + 1) * P],
                        h[:, cc * CH:(cc + 1) * CH],
                        start=(c == 0),
                        stop=(c == n_in_g - 1),
                    )
                    NAME_INFO[mm4.ins.name] = (s, f"mm4.{cc}")
                    if c == n_in_g - 1:
                        rows = n_in_g * RG
                        k0 = k - (n_in_g - 1)
                        ob = outsp.tile([rows, CH], f32, tag="ob")
                        blcol = btile[:rows, 3:4]
                        fe = "act" if plan_t["act"] / cfg["act_share"] <= \
                            plan_t["dve"] / (1 - cfg["act_share"]) else "dve"
                        plan_t[fe] += op_cost(fe, CH)
                        if fe == "act":
                            fin = nc.scalar.activation(
                                ob[:, :], ps4[:rows, :], Act.Identity,
                                bias=blcol, scale=1.0,
                            )
                        else:
                            fin = nc.vector.tensor_scalar(
                                ob[:, :], ps4[:rows, :], blcol, None, Alu.add
                            )
                        NAME_INFO[fin.ins.name] = (s, "final")
                        nc.sync.dma_start(
                            out_r[:, k0:k0 + n_in_g, :],
                            ob[:, :],
                        )

            # wavefront emission: slab s stage k at tick s+k, so DMA issues
            # (stage 0 of later slabs) precede older slabs' L4 reads of w4t
            n_slabs = len(slabs)
            for tick in range(n_slabs + 3):
                for s in range(max(0, tick - 3), min(tick + 1, n_slabs)):
                    emit_stage(s, tick - s)

    import concourse.mybir as mybir_

    # Move SEQ-blocking EventSemaphore waits onto adjacent engine
    # instructions so they park in the engine wait queue instead of stalling
    # the sequencer (engine execution is in-order, so gating instruction i
    # still gates everything behind it).
    eng_ok = {}
    if False:
        eng_ok = {
            mybir_.EngineType.Activation: (mybir_.InstActivation,),
            mybir_.EngineType.DVE: (mybir_.InstTensorScalarPtr,
                                    mybir_.InstMemset),
        }
    for blk in nc.main_func.blocks:
        insts = blk.instructions
        idx = 0
        while idx < len(insts):
            inst = insts[idx]
            if (isinstance(inst, mybir_.InstEventSemaphore)
                    and inst.engine in eng_ok
                    and inst.sync_info is not None
                    and len(inst.sync_info.on_wait) > 0
                    and len(inst.sync_info.on_update) == 0):
                types = eng_ok[inst.engine]
                target = None
                j = idx + 1
                while j < len(insts) and j <= idx + 12:
                    if (isinstance(insts[j], types)
                            and insts[j].engine == inst.engine):
                        target = insts[j]
                        break
                    j += 1
                if target is not None:
                    si = target.sync_info
                    if si is None:
                        target.sync_info = mybir_.SyncInfo(
                            on_wait=list(inst.sync_info.on_wait), on_update=[]
                        )
                    else:
                        si.on_wait = list(inst.sync_info.on_wait) + \
                            list(si.on_wait)
                    insts.pop(idx)
                    continue
            idx += 1

    # Walrus codegen cannot reliably attach semaphore waits to self-loading
    # matmuls; hoist every matmul's waits onto a PE nop inserted just before
    # it (sequencer-side wait, same semantics).
    for blk in nc.main_func.blocks:
        insts = blk.instructions
        idx = 0
        while idx < len(insts):
            inst = insts[idx]
            if isinstance(inst, mybir_.InstMatmult):
                si = inst.sync_info
                if si is not None and len(si.on_wait) > 0:
                    nop = mybir_.InstNoOp(
                        name=nc.get_next_instruction_name(), ins=[], outs=[]
                    )
                    nop.engine = inst.engine
                    nop.bass_nofuse = True
                    nop.sync_info = mybir_.SyncInfo(on_wait=si.on_wait, on_update=[])
                    si.on_wait = []
                    nc.register_instruction(nop)
                    insts.insert(idx, nop)
                    idx += 1
            idx += 1

    for blk in nc.main_func.blocks:
        for inst in blk.instructions:
            if isinstance(inst, mybir_.InstMatmult):
                si = inst.sync_info
                assert si is None or len(si.on_wait) == 0, inst.name

    nc.compile()
    return nc


def _blockdiag4(wT):
    o = np.zeros((P, P), dtype=np.float32)
    for b in range(RG):
        o[32 * b:32 * b + 32, 32 * b:32 * b + 32] = wT
    return o


def _prep_host_inputs(z, w1, b1, w2, b2, w3, b3, wl, bl):
    nacc = max(CFG["l4_groups"])
    f32 = np.float32
    b1e = (b1 + w1[:, C:] @ z[0]).astype(f32)

    w13 = np.concatenate(
        [_blockdiag4(w1[:, :C].T), _blockdiag4(w2.T), _blockdiag4(w3.T)],
        axis=1,
    ).astype(BF16_NP)

    w4s = np.zeros((P, nacc * P), dtype=f32)
    for c in range(nacc):
        for m in range(RG):
            w4s[32 * m:32 * m + 32, c * P + m * nacc + c] = wl[0, :]
    w4s = w4s.astype(BF16_NP)

    bias = np.zeros((P, 4), dtype=f32)
    bias[:, 0] = np.tile(b1e, RG)
    bias[:, 1] = np.tile(b2.astype(f32), RG)
    bias[:, 2] = np.tile(b3.astype(f32), RG)
    bias[:, 3] = f32(bl[0])
    return w13, w4s, bias


def _restripe(shard):
    """[32, npix] channel-major shard -> [128, npix/4] (block, channel) rows."""
    npix = shard.shape[1]
    return np.ascontiguousarray(
        shard.reshape(C, RG, npix // RG).transpose(1, 0, 2).reshape(P, npix // RG)
    ).astype(BF16_NP)


_NC_CACHE = {}


def _run(feature_map, z, w1, b1, w2, b2, w3, b3, wl, bl, **spmd_kwargs):
    from concourse.bass_utils import run_bass_kernel_spmd

    feature_map = np.asarray(feature_map, dtype=np.float32)
    z = np.asarray(z, dtype=np.float32)
    w1, b1 = np.asarray(w1, np.float32), np.asarray(b1, np.float32)
    w2, b2 = np.asarray(w2, np.float32), np.asarray(b2, np.float32)
    w3, b3 = np.asarray(w3, np.float32), np.asarray(b3, np.float32)
    wl, bl = np.asarray(wl, np.float32), np.asarray(bl, np.float32)

    w13, w4s, bias = _prep_host_inputs(z, w1, b1, w2, b2, w3, b3, wl, bl)

    fm_flat = feature_map.reshape(C, VOL)
    in_maps = []
    for k in range(NCORES):
        shard = _restripe(fm_flat[:, k * NPIX:(k + 1) * NPIX])
        in_maps.append({"fm": shard, "w13": w13, "w4s": w4s, "bias": bias})

    if "nc" not in _NC_CACHE:
        _NC_CACHE["nc"] = _build_nc()
    nc = _NC_CACHE["nc"]

    res = run_bass_kernel_spmd(nc, in_maps, core_ids=list(range(NCORES)), **spmd_kwargs)
    out = np.empty((VOL,), dtype=np.float32)
    for k in range(NCORES):
        out[k * NPIX:(k + 1) * NPIX] = res.results[k]["out"]
    return out.reshape(1, 1, 96, 96, 96), res


def kernel(feature_map, z, w1, b1, w2, b2, w3, b3, wl, bl):
    out, _ = _run(feature_map, z, w1, b1, w2, b2, w3, b3, wl, bl)
    return out


# revision 8
# speedup vs baseline: 1.1136x; 1.0149x over previous
"""Trainium2 Bass kernel for nn_FComb_79319456023150 (dense_cnn).

Per-pixel MLP over a 96^3 volume: four 1x1x1 convs (38->32->32->32->1 channels
with relu between). z is batch-constant, so w1[:, 32:38] @ z folds into the
layer-1 bias and every layer becomes a K=32 channel GEMM.

Sharding: spatial (outermost X axis) across 8 cores, 110592 pixels each.
Weights/biases replicated.

Device layout per core: the host restripes each shard to [128, 27648] =
4 pixel-blocks x 32 channels on partitions, pixels on the free dim, cast to
bf16 (halves HBM traffic; fp32 PSUM accumulation keeps rel-err ~5e-3).
Layers 1-3 use a block-diagonal [128, 128] bf16 weight (4 copies of W^T on
the diagonal), so one matmul applies the 32x32 GEMM to 4 pixel blocks at
once. Work is split into 512-column slabs cycling over SEVEN single-bank
PSUM regions; the eighth bank accumulates layer 4: chunk c's sparse [128,128]
weight lands wl @ h3 on psum rows 4c+m, so one wide [rows, 512] crossing
evacuates an entire accumulation group and batched affine DMAs ship it.

Relu rides the mandatory PSUM->SBUF crossing as one whole-crossing op per
slab-layer, alternating ScalarE (activation Relu w/ bias) and VectorE (fused
tensor_scalar add+max) via a planner that equalizes modeled engine time.
Seven regions keep ~6 crossings in flight so both crossing engines stay
~85% busy, which is the throughput bound for this kernel.
"""

import sys

import numpy as np

if "/opt/trn_rl_repo" not in sys.path:
    sys.path.insert(0, "/opt/trn_rl_repo")

import ml_dtypes

C = 32
P = 128
RG = 4
CH = 512
VOL = 96 * 96 * 96
NCORES = 8
NPIX = VOL // NCORES       # 110592
FREE = NPIX // RG          # 27648
NCHUNKS = FREE // CH       # 54

BF16_NP = ml_dtypes.bfloat16

CFG = dict(
    l4_groups=(18, 18, 18),
    dma_targets=(512, 1024, 2048, 3072),
    dma_steady=3584,
    slab_pattern=(512,) * 7,
    act_share=0.505,
    attach_waits=False,
)

NAME_INFO = {}


def _slabs():
    pat = CFG["slab_pattern"]
    out = []
    col, i = 0, 0
    while col < FREE:
        r = i % len(pat)
        w = min(pat[r], FREE - col)
        out.append((col, w, r))
        col += w
        i += 1
    return out


def _build_nc(npix=NPIX):
    import concourse.mybir as mybir
    from concourse import bacc
    from concourse.tile import TileContext

    cfg = CFG
    l4_groups = cfg["l4_groups"]
    assert sum(l4_groups) == NCHUNKS and max(l4_groups) * RG <= P
    # m-major psum rows (q = m*nacc + c) require uniform group sizes
    assert len(set(l4_groups)) == 1

    f32 = mybir.dt.float32
    bf16 = mybir.dt.bfloat16
    Alu = mybir.AluOpType
    Act = mybir.ActivationFunctionType

    free = npix // RG
    assert free == FREE

    nacc_max = max(l4_groups)

    nc = bacc.Bacc()
    fm = nc.dram_tensor("fm", [P, free], bf16, kind="ExternalInput")
    w13 = nc.dram_tensor("w13", [P, 3 * P], bf16, kind="ExternalInput")
    w4s = nc.dram_tensor("w4s", [P, nacc_max * P], bf16, kind="ExternalInput")
    bias = nc.dram_tensor("bias", [P, 4], f32, kind="ExternalInput")
    out = nc.dram_tensor("out", [npix], f32, kind="ExternalOutput")

    out_r = out.rearrange("(m k n) -> m k n", m=RG, k=NCHUNKS, n=CH)

    slabs = _slabs()

    # Input DMA groups: consecutive slabs batched to ramped byte targets.
    dma_groups = []
    targets = list(cfg["dma_targets"])
    cur, cw, ti = [], 0, 0
    for idx, (c0, w, r) in enumerate(slabs):
        cur.append(idx)
        cw += w
        tgt = targets[ti] if ti < len(targets) else cfg["dma_steady"]
        if cw >= tgt:
            dma_groups.append(cur)
            cur, cw = [], 0
            ti += 1
    if cur:
        dma_groups.append(cur)
    group_of_slab = {}
    for gi, g in enumerate(dma_groups):
        for idx in g:
            group_of_slab[idx] = gi

    l4_of_chunk = {}
    k = 0
    for g, n in enumerate(l4_groups):
        for c in range(n):
            l4_of_chunk[k] = (g, c, n)
            k += 1

    # crossing engine planner: stream-local alternation, seeded to equalize
    # modeled engine time (ACT is faster per op; DVE runs at 0.96 GHz)
    plan_t = {"act": 0.0, "dve": 0.0}

    def op_cost(eng, fr):
        return (fr + 222) / 1.2 if eng == "act" else (fr + 120) / 0.96

    seed_n = [0]

    def pick_seed(fr):
        pol = cfg.get("seed_policy", "greedy")
        if pol == "slab_parity":
            first = "act" if seed_n[0] % 2 == 0 else "dve"
            seed_n[0] += 1
            seq = [first, ("dve" if first == "act" else "act"), first]
            for e in seq:
                plan_t[e] += op_cost(e, fr)
            return seq
        best, arg = None, None
        for first in ("act", "dve"):
            seq = [first, ("dve" if first == "act" else "act"), first]
            t = dict(plan_t)
            for e in seq:
                t[e] += op_cost(e, fr)
            m = max(t["act"] / cfg["act_share"],
                    t["dve"] / (1 - cfg["act_share"]))
            if best is None or m < best:
                best, arg = m, seq
        for e in arg:
            plan_t[e] += op_cost(e, fr)
        return arg

    with TileContext(nc) as tc:
        with (
            tc.tile_pool(name="const", bufs=1) as constp,
            tc.tile_pool(name="data", bufs=4) as datap,
            tc.tile_pool(name="acts", bufs=4) as actp,
            tc.tile_pool(name="outs", bufs=2) as outsp,
            tc.tile_pool(name="psb", bufs=1, space="PSUM") as psb,
        ):
            w13t = constp.tile([P, 3 * P], bf16)
            btile = constp.tile([P, 4], f32)
            w4t = constp.tile([P, nacc_max * P], bf16)

            # dummy relu at t~0 pulls the ACT table load off the ramp
            tiny = constp.tile([1, 2], f32)
            tiny2 = constp.tile([1, 2], f32)
            nc.vector.memset(tiny[:, :], 0.0)
            pre = nc.scalar.activation(
                tiny2[:, :], tiny[:, :], Act.Relu, bias=0.0, scale=1.0
            )
            NAME_INFO[pre.ins.name] = (-1, "preload")

            w4_half = nacc_max * P // 2
            w4_loads = [(0, w4_half), (w4_half, nacc_max * P)]

            xtiles = {}
            ob = None
            ps4 = None
            fired_const = False
            slab_h = {}
            slab_engs = {}

            def emit_stage(s, stage):
                nonlocal ob, ps4, fired_const
                c0, w, r = slabs[s]
                nch = w // CH
                if stage == 0:
                    gi = group_of_slab[s]
                    if gi not in xtiles:
                        gslabs = dma_groups[gi]
                        ga = slabs[gslabs[0]][0]
                        gb = slabs[gslabs[-1]][0] + slabs[gslabs[-1]][1]
                        xt = datap.tile([P, gb - ga], bf16, tag="x")
                        nc.sync.dma_start(xt, fm[:, ga:gb])
                        xtiles[gi] = (xt, ga)
                        if not fired_const:
                            nc.sync.dma_start(w13t, w13[:, :])
                            nc.sync.dma_start(btile, bias[:, :])
                            fired_const = True
                        elif gi in (1, 2) and w4_loads:
                            a, b = w4_loads.pop(0)
                            nc.sync.dma_start(w4t[:, a:b], w4s[:, a:b])
                    if s >= 3 and w4_loads:
                        a, b = w4_loads.pop(0)
                        nc.sync.dma_start(w4t[:, a:b], w4s[:, a:b])
                    xt, ga = xtiles[group_of_slab[s]]
                    slab_h[s] = xt[:, c0 - ga:c0 - ga + w]
                    slab_engs[s] = pick_seed(w)

                h = slab_h[s]
                if stage < 3:
                    layer = stage
                    ps = psb.tile([P, w], f32, tag=f"ps{r}")
                    wsl = w13t[:, layer * P:(layer + 1) * P]
                    for cc in range(nch):
                        mm = nc.tensor.matmul(
                            ps[:, cc * CH:(cc + 1) * CH],
                            wsl,
                            h[:, cc * CH:(cc + 1) * CH],
                            start=True,
                            stop=True,
                        )
                        NAME_INFO[mm.ins.name] = (s, f"mm{layer}.{cc}")
                    hn = actp.tile([P, w], bf16, tag=f"h{r}{layer}")
                    bcol = btile[:, layer:layer + 1]
                    if slab_engs[s][layer] == "act":
                        xop = nc.scalar.activation(
                            hn[:, :], ps[:, :], Act.Relu, bias=bcol, scale=1.0
                        )
                    else:
                        xop = nc.vector.tensor_scalar(
                            hn[:, :], ps[:, :], bcol, 0.0, Alu.add, Alu.max
                        )
                    NAME_INFO[xop.ins.name] = (s, f"relu{layer}")
                    slab_h[s] = hn
                    return

                # stage 3: layer 4, accumulate into the shared bank
                for cc in range(nch):
                    k = c0 // CH + cc
                    g, c, n_in_g = l4_of_chunk[k]
                    if c == 0:
                        ps4 = psb.tile([P, CH], f32, tag="psL4")
                    mm4 = nc.tensor.matmul(
                        ps4[:, :],
                        w4t[:, c * P:(c + 1) * P],
                        h[:, cc * CH:(cc + 1) * CH],
                        start=(c == 0),
                        stop=(c == n_in_g - 1),
                    )
                    NAME_INFO[mm4.ins.name] = (s, f"mm4.{cc}")
                    if c == n_in_g - 1:
                        rows = n_in_g * RG
                        k0 = k - (n_in_g - 1)
                        ob = outsp.tile([rows, CH], f32, tag="ob")
                        blcol = btile[:rows, 3:4]
                        fe = "act" if plan_t["act"] / cfg["act_share"] <= \
                            plan_t["dve"] / (1 - cfg["act_share"]) else "dve"
                        plan_t[fe] += op_cost(fe, CH)
                        if fe == "act":
                            fin = nc.scalar.activation(
                                ob[:, :], ps4[:rows, :], Act.Identity,
                                bias=blcol, scale=1.0,
                            )
                        else:
                            fin = nc.vector.tensor_scalar(
                                ob[:, :], ps4[:rows, :], blcol, None, Alu.add
                            )
                        NAME_INFO[fin.ins.name] = (s, "final")
                        nc.sync.dma_start(
                            out_r[:, k0:k0 + n_in_g, :],
                            ob[:, :],
                        )

            # wavefront emission: slab s stage k at tick s+k, so DMA issues
            # (stage 0 of later slabs) precede older slabs' L4 reads of w4t
            n_slabs = len(slabs)
            for tick in range(n_slabs + 3):
                for s in range(max(0, tick - 3), min(tick + 1, n_slabs)):
                    emit_stage(s, tick - s)

    import concourse.mybir as mybir_

    # Move SEQ-blocking EventSemaphore waits onto adjacent engine
    # instructions so they park in the engine wait queue instead of stalling
    # the sequencer (engine execution is in-order, so gating instruction i
    # still gates everything behind it).
    eng_ok = {}
    if CFG.get("attach_waits"):
        eng_ok = {
            mybir_.EngineType.Activation: (mybir_.InstActivation,),
            mybir_.EngineType.DVE: (mybir_.InstTensorScalarPtr,
                                    mybir_.InstMemset),
        }
    for blk in nc.main_func.blocks:
        insts = blk.instructions
        idx = 0
        while idx < len(insts):
            inst = insts[idx]
            if (isinstance(inst, mybir_.InstEventSemaphore)
                    and inst.engine in eng_ok
                    and inst.sync_info is not None
                    and len(inst.sync_info.on_wait) > 0
                    and len(inst.sync_info.on_update) == 0):
                types = eng_ok[inst.engine]
                target = None
                j = idx + 1
                while j < len(insts) and j <= idx + 12:
                    if (isinstance(insts[j], types)
                            and insts[j].engine == inst.engine):
                        target = insts[j]
                        break
                    j += 1
                if target is not None:
                    si = target.sync_info
                    if si is None:
                        target.sync_info = mybir_.SyncInfo(
                            on_wait=list(inst.sync_info.on_wait), on_update=[]
                        )
                    else:
                        si.on_wait = list(inst.sync_info.on_wait) + \
                            list(si.on_wait)
                    insts.pop(idx)
                    continue
            idx += 1

    # Walrus codegen cannot reliably attach semaphore waits to self-loading
    # matmuls; hoist every matmul's waits onto a PE nop inserted just before
    # it (sequencer-side wait, same semantics).
    for blk in nc.main_func.blocks:
        insts = blk.instructions
        idx = 0
        while idx < len(insts):
            inst = insts[idx]
            if isinstance(inst, mybir_.InstMatmult):
                si = inst.sync_info
                if si is not None and len(si.on_wait) > 0:
                    nop = mybir_.InstNoOp(
                        name=nc.get_next_instruction_name(), ins=[], outs=[]
                    )
                    nop.engine = inst.engine
                    nop.bass_nofuse = True
                    nop.sync_info = mybir_.SyncInfo(on_wait=si.on_wait, on_update=[])
                    si.on_wait = []
                    nc.register_instruction(nop)
                    insts.insert(idx, nop)
                    idx += 1
            idx += 1

    for blk in nc.main_func.blocks:
        for inst in blk.instructions:
            if isinstance(inst, mybir_.InstMatmult):
                si = inst.sync_info
                assert si is None or len(si.on_wait) == 0, inst.name

    nc.compile()
    return nc


def _blockdiag4(wT):
    o = np.zeros((P, P), dtype=np.float32)
    for b in range(RG):
        o[32 * b:32 * b + 32, 32 * b:32 * b + 32] = wT
    return o


def _prep_host_inputs(z, w1, b1, w2, b2, w3, b3, wl, bl):
    nacc = max(CFG["l4_groups"])
    f32 = np.float32
    b1e = (b1 + w1[:, C:] @ z[0]).astype(f32)

    w13 = np.concatenate(
        [_blockdiag4(w1[:, :C].T), _blockdiag4(w2.T), _blockdiag4(w3.T)],
        axis=1,
    ).astype(BF16_NP)

    w4s = np.zeros((P, nacc * P), dtype=f32)
    for c in range(nacc):
        for m in range(RG):
            w4s[32 * m:32 * m + 32, c * P + m * nacc + c] = wl[0, :]
    w4s = w4s.astype(BF16_NP)

    bias = np.zeros((P, 4), dtype=f32)
    bias[:, 0] = np.tile(b1e, RG)
    bias[:, 1] = np.tile(b2.astype(f32), RG)
    bias[:, 2] = np.tile(b3.astype(f32), RG)
    bias[:, 3] = f32(bl[0])
    return w13, w4s, bias


def _restripe(shard):
    """[32, npix] channel-major shard -> [128, npix/4] (block, channel) rows."""
    npix = shard.shape[1]
    return np.ascontiguousarray(
        shard.reshape(C, RG, npix // RG).transpose(1, 0, 2).reshape(P, npix // RG)
    ).astype(BF16_NP)


_NC_CACHE = {}


def _run(feature_map, z, w1, b1, w2, b2, w3, b3, wl, bl, **spmd_kwargs):
    from concourse.bass_utils import run_bass_kernel_spmd

    feature_map = np.asarray(feature_map, dtype=np.float32)
    z = np.asarray(z, dtype=np.float32)
    w1, b1 = np.asarray(w1, np.float32), np.asarray(b1, np.float32)
    w2, b2 = np.asarray(w2, np.float32), np.asarray(b2, np.float32)
    w3, b3 = np.asarray(w3, np.float32), np.asarray(b3, np.float32)
    wl, bl = np.asarray(wl, np.float32), np.asarray(bl, np.float32)

    w13, w4s, bias = _prep_host_inputs(z, w1, b1, w2, b2, w3, b3, wl, bl)

    fm_flat = feature_map.reshape(C, VOL)
    in_maps = []
    for k in range(NCORES):
        shard = _restripe(fm_flat[:, k * NPIX:(k + 1) * NPIX])
        in_maps.append({"fm": shard, "w13": w13, "w4s": w4s, "bias": bias})

    if "nc" not in _NC_CACHE:
        _NC_CACHE["nc"] = _build_nc()
    nc = _NC_CACHE["nc"]

    res = run_bass_kernel_spmd(nc, in_maps, core_ids=list(range(NCORES)), **spmd_kwargs)
    out = np.empty((VOL,), dtype=np.float32)
    for k in range(NCORES):
        out[k * NPIX:(k + 1) * NPIX] = res.results[k]["out"]
    return out.reshape(1, 1, 96, 96, 96), res


def kernel(feature_map, z, w1, b1, w2, b2, w3, b3, wl, bl):
    out, _ = _run(feature_map, z, w1, b1, w2, b2, w3, b3, wl, bl)
    return out


# revision 9
# speedup vs baseline: 1.1174x; 1.0034x over previous
"""Trainium2 Bass kernel for nn_FComb_79319456023150 (dense_cnn).

Per-pixel MLP over a 96^3 volume: four 1x1x1 convs (38->32->32->32->1 channels
with relu between). z is batch-constant, so w1[:, 32:38] @ z folds into the
layer-1 bias and every layer becomes a K=32 channel GEMM.

Sharding: spatial (outermost X axis) across 8 cores, 110592 pixels each.
Weights/biases replicated.

Device layout per core: the host restripes each shard to [128, 27648] =
4 pixel-blocks x 32 channels on partitions, pixels on the free dim, cast to
bf16 (halves HBM traffic; fp32 PSUM accumulation keeps rel-err ~5e-3).
Layers 1-3 use a block-diagonal [128, 128] bf16 weight (4 copies of W^T on
the diagonal), so one matmul applies the 32x32 GEMM to 4 pixel blocks at
once. Work is split into 512-column slabs cycling over SEVEN single-bank
PSUM regions; the eighth bank accumulates layer 4: chunk c's sparse [128,128]
weight lands wl @ h3 on psum rows 4c+m, so one wide [rows, 512] crossing
evacuates an entire accumulation group and batched affine DMAs ship it.

Relu rides the mandatory PSUM->SBUF crossing as one whole-crossing op per
slab-layer, alternating ScalarE (activation Relu w/ bias) and VectorE (fused
tensor_scalar add+max) via a planner that equalizes modeled engine time.
Seven regions keep ~6 crossings in flight so both crossing engines stay
~85% busy, which is the throughput bound for this kernel.
"""

import sys

import numpy as np

if "/opt/trn_rl_repo" not in sys.path:
    sys.path.insert(0, "/opt/trn_rl_repo")

import ml_dtypes

C = 32
P = 128
RG = 4
CH = 512
VOL = 96 * 96 * 96
NCORES = 8
NPIX = VOL // NCORES       # 110592
FREE = NPIX // RG          # 27648
NCHUNKS = FREE // CH       # 54

BF16_NP = ml_dtypes.bfloat16

CFG = dict(
    l4_groups=(18, 18, 18),
    dma_targets=(512, 1024, 2048, 3072),
    dma_steady=3584,
    slab_pattern=(512,) * 7,
    act_share=0.505,
    attach_waits=False,
)

NAME_INFO = {}


def _slabs():
    pat = CFG["slab_pattern"]
    out = []
    col, i = 0, 0
    while col < FREE:
        r = i % len(pat)
        w = min(pat[r], FREE - col)
        out.append((col, w, r))
        col += w
        i += 1
    return out


def _build_nc(npix=NPIX):
    import concourse.mybir as mybir
    from concourse import bacc
    from concourse.tile import TileContext

    cfg = CFG
    l4_groups = cfg["l4_groups"]
    assert sum(l4_groups) == NCHUNKS and max(l4_groups) * RG <= P
    # m-major psum rows (q = m*nacc + c) require uniform group sizes
    assert len(set(l4_groups)) == 1

    f32 = mybir.dt.float32
    bf16 = mybir.dt.bfloat16
    Alu = mybir.AluOpType
    Act = mybir.ActivationFunctionType

    free = npix // RG
    assert free == FREE

    nacc_max = max(l4_groups)

    nc = bacc.Bacc()
    fm = nc.dram_tensor("fm", [P, free], bf16, kind="ExternalInput")
    w13 = nc.dram_tensor("w13", [P, 3 * P], bf16, kind="ExternalInput")
    w4s = nc.dram_tensor("w4s", [P, nacc_max * P], bf16, kind="ExternalInput")
    bias = nc.dram_tensor("bias", [P, 4], f32, kind="ExternalInput")
    out = nc.dram_tensor("out", [npix], f32, kind="ExternalOutput")

    out_r = out.rearrange("(m k n) -> m k n", m=RG, k=NCHUNKS, n=CH)

    slabs = _slabs()

    # Input DMA groups: consecutive slabs batched to ramped byte targets.
    dma_groups = []
    targets = list(cfg["dma_targets"])
    cur, cw, ti = [], 0, 0
    for idx, (c0, w, r) in enumerate(slabs):
        cur.append(idx)
        cw += w
        tgt = targets[ti] if ti < len(targets) else cfg["dma_steady"]
        if cw >= tgt:
            dma_groups.append(cur)
            cur, cw = [], 0
            ti += 1
    if cur:
        dma_groups.append(cur)
    group_of_slab = {}
    for gi, g in enumerate(dma_groups):
        for idx in g:
            group_of_slab[idx] = gi

    l4_of_chunk = {}
    k = 0
    for g, n in enumerate(l4_groups):
        for c in range(n):
            l4_of_chunk[k] = (g, c, n)
            k += 1

    # crossing engine planner: stream-local alternation, seeded to equalize
    # modeled engine time (ACT is faster per op; DVE runs at 0.96 GHz)
    plan_t = {"act": 0.0, "dve": 0.0}

    def op_cost(eng, fr):
        return (fr + 222) / 1.2 if eng == "act" else (fr + 120) / 0.96

    seed_n = [0]

    def pick_seed(fr):
        pol = cfg.get("seed_policy", "greedy")
        if pol == "slab_parity":
            first = "act" if seed_n[0] % 2 == 0 else "dve"
            seed_n[0] += 1
            seq = [first, ("dve" if first == "act" else "act"), first]
            for e in seq:
                plan_t[e] += op_cost(e, fr)
            return seq
        best, arg = None, None
        for first in ("act", "dve"):
            seq = [first, ("dve" if first == "act" else "act"), first]
            t = dict(plan_t)
            for e in seq:
                t[e] += op_cost(e, fr)
            m = max(t["act"] / cfg["act_share"],
                    t["dve"] / (1 - cfg["act_share"]))
            if best is None or m < best:
                best, arg = m, seq
        for e in arg:
            plan_t[e] += op_cost(e, fr)
        return arg

    with TileContext(nc) as tc:
        with (
            tc.tile_pool(name="const", bufs=1) as constp,
            tc.tile_pool(name="data", bufs=4) as datap,
            tc.tile_pool(name="acts", bufs=4) as actp,
            tc.tile_pool(name="outs", bufs=2) as outsp,
            tc.tile_pool(name="psb", bufs=1, space="PSUM") as psb,
        ):
            w13t = constp.tile([P, 3 * P], bf16)
            btile = constp.tile([P, 4], f32)
            w4t = constp.tile([P, nacc_max * P], bf16)

            # dummy relu at t~0 pulls the ACT table load off the ramp
            tiny = constp.tile([1, 2], f32)
            tiny2 = constp.tile([1, 2], f32)
            nc.vector.memset(tiny[:, :], 0.0)
            pre = nc.scalar.activation(
                tiny2[:, :], tiny[:, :], Act.Relu, bias=0.0, scale=1.0
            )
            NAME_INFO[pre.ins.name] = (-1, "preload")

            w4_half = nacc_max * P // 2
            w4_loads = [(0, w4_half), (w4_half, nacc_max * P)]

            xtiles = {}
            ob = None
            ps4 = None
            fired_const = False
            slab_h = {}
            slab_engs = {}

            def emit_stage(s, stage):
                nonlocal ob, ps4, fired_const
                c0, w, r = slabs[s]
                nch = w // CH
                if stage == 0:
                    gi = group_of_slab[s]
                    if gi not in xtiles:
                        gslabs = dma_groups[gi]
                        ga = slabs[gslabs[0]][0]
                        gb = slabs[gslabs[-1]][0] + slabs[gslabs[-1]][1]
                        xt = datap.tile([P, gb - ga], bf16, tag="x")
                        nc.sync.dma_start(xt, fm[:, ga:gb])
                        xtiles[gi] = (xt, ga)
                        if not fired_const:
                            nc.sync.dma_start(w13t, w13[:, :])
                            nc.sync.dma_start(btile, bias[:, :])
                            fired_const = True
                        elif gi in (1, 2) and w4_loads:
                            a, b = w4_loads.pop(0)
                            nc.sync.dma_start(w4t[:, a:b], w4s[:, a:b])
                    if s >= 3 and w4_loads:
                        a, b = w4_loads.pop(0)
                        nc.sync.dma_start(w4t[:, a:b], w4s[:, a:b])
                    xt, ga = xtiles[group_of_slab[s]]
                    slab_h[s] = xt[:, c0 - ga:c0 - ga + w]
                    slab_engs[s] = pick_seed(w)

                h = slab_h[s]
                if stage < 3:
                    layer = stage
                    ps = psb.tile([P, w], f32, tag=f"ps{r}")
                    wsl = w13t[:, layer * P:(layer + 1) * P]
                    for cc in range(nch):
                        mm = nc.tensor.matmul(
                            ps[:, cc * CH:(cc + 1) * CH],
                            wsl,
                            h[:, cc * CH:(cc + 1) * CH],
                            start=True,
                            stop=True,
                        )
                        NAME_INFO[mm.ins.name] = (s, f"mm{layer}.{cc}")
                    hn = actp.tile([P, w], bf16, tag=f"h{r}{layer}")
                    bcol = btile[:, layer:layer + 1]
                    # any-engine: the tile scheduler load-balances the
                    # crossing between ScalarE and VectorE
                    xop = nc.any.tensor_scalar(
                        hn[:, :], ps[:, :], bcol, 0.0, Alu.add, Alu.max
                    )
                    NAME_INFO[xop.ins.name] = (s, f"relu{layer}")
                    slab_h[s] = hn
                    return

                # stage 3: layer 4, accumulate into the shared bank
                for cc in range(nch):
                    k = c0 // CH + cc
                    g, c, n_in_g = l4_of_chunk[k]
                    if c == 0:
                        ps4 = psb.tile([P, CH], f32, tag="psL4")
                    mm4 = nc.tensor.matmul(
                        ps4[:, :],
                        w4t[:, c * P:(c + 1) * P],
                        h[:, cc * CH:(cc + 1) * CH],
                        start=(c == 0),
                        stop=(c == n_in_g - 1),
                    )
                    NAME_INFO[mm4.ins.name] = (s, f"mm4.{cc}")
                    if c == n_in_g - 1:
                        rows = n_in_g * RG
                        k0 = k - (n_in_g - 1)
                        ob = outsp.tile([rows, CH], f32, tag="ob")
                        blcol = btile[:rows, 3:4]
                        fin = nc.any.tensor_scalar(
                            ob[:, :], ps4[:rows, :], blcol, None, Alu.add
                        )
                        NAME_INFO[fin.ins.name] = (s, "final")
                        nc.sync.dma_start(
                            out_r[:, k0:k0 + n_in_g, :],
                            ob[:, :],
                        )

            # wavefront emission: slab s stage k at tick s+k, so DMA issues
            # (stage 0 of later slabs) precede older slabs' L4 reads of w4t
            n_slabs = len(slabs)
            for tick in range(n_slabs + 3):
                for s in range(max(0, tick - 3), min(tick + 1, n_slabs)):
                    emit_stage(s, tick - s)

    import concourse.mybir as mybir_

    # Move SEQ-blocking EventSemaphore waits onto adjacent engine
    # instructions so they park in the engine wait queue instead of stalling
    # the sequencer (engine execution is in-order, so gating instruction i
    # still gates everything behind it).
    eng_ok = {}
    if CFG.get("attach_waits"):
        eng_ok = {
            mybir_.EngineType.Activation: (mybir_.InstActivation,),
            mybir_.EngineType.DVE: (mybir_.InstTensorScalarPtr,
                                    mybir_.InstMemset),
        }
    for blk in nc.main_func.blocks:
        insts = blk.instructions
        idx = 0
        while idx < len(insts):
            inst = insts[idx]
            if (isinstance(inst, mybir_.InstEventSemaphore)
                    and inst.engine in eng_ok
                    and inst.sync_info is not None
                    and len(inst.sync_info.on_wait) > 0
                    and len(inst.sync_info.on_update) == 0):
                types = eng_ok[inst.engine]
                target = None
                j = idx + 1
                while j < len(insts) and j <= idx + 12:
                    if (isinstance(insts[j], types)
                            and insts[j].engine == inst.engine):
                        target = insts[j]
                        break
                    j += 1
                if target is not None:
                    si = target.sync_info
                    if si is None:
                        target.sync_info = mybir_.SyncInfo(
                            on_wait=list(inst.sync_info.on_wait), on_update=[]
                        )
                    else:
                        si.on_wait = list(inst.sync_info.on_wait) + \
                            list(si.on_wait)
                    insts.pop(idx)
                    continue
            idx += 1

    # Walrus codegen cannot reliably attach semaphore waits to self-loading
    # matmuls; hoist every matmul's waits onto a PE nop inserted just before
    # it (sequencer-side wait, same semantics).
    for blk in nc.main_func.blocks:
        insts = blk.instructions
        idx = 0
        while idx < len(insts):
            inst = insts[idx]
            if isinstance(inst, mybir_.InstMatmult):
                si = inst.sync_info
                if si is not None and len(si.on_wait) > 0:
                    nop = mybir_.InstNoOp(
                        name=nc.get_next_instruction_name(), ins=[], outs=[]
                    )
                    nop.engine = inst.engine
                    nop.bass_nofuse = True
                    nop.sync_info = mybir_.SyncInfo(on_wait=si.on_wait, on_update=[])
                    si.on_wait = []
                    nc.register_instruction(nop)
                    insts.insert(idx, nop)
                    idx += 1
            idx += 1

    for blk in nc.main_func.blocks:
        for inst in blk.instructions:
            if isinstance(inst, mybir_.InstMatmult):
                si = inst.sync_info
                assert si is None or len(si.on_wait) == 0, inst.name

    nc.compile()
    return nc


def _blockdiag4(wT):
    o = np.zeros((P, P), dtype=np.float32)
    for b in range(RG):
        o[32 * b:32 * b + 32, 32 * b:32 * b + 32] = wT
    return o


def _prep_host_inputs(z, w1, b1, w2, b2, w3, b3, wl, bl):
    nacc = max(CFG["l4_groups"])
    f32 = np.float32
    b1e = (b1 + w1[:, C:] @ z[0]).astype(f32)

    w13 = np.concatenate(
        [_blockdiag4(w1[:, :C].T), _blockdiag4(w2.T), _blockdiag4(w3.T)],
        axis=1,
    ).astype(BF16_NP)

    w4s = np.zeros((P, nacc * P), dtype=f32)
    for c in range(nacc):
        for m in range(RG):
            w4s[32 * m:32 * m + 32, c * P + m * nacc + c] = wl[0, :]
    w4s = w4s.astype(BF16_NP)

    bias = np.zeros((P, 4), dtype=f32)
    bias[:, 0] = np.tile(b1e, RG)
    bias[:, 1] = np.tile(b2.astype(f32), RG)
    bias[:, 2] = np.tile(b3.astype(f32), RG)
    bias[:, 3] = f32(bl[0])
    return w13, w4s, bias


def _restripe(shard):
    """[32, npix] channel-major shard -> [128, npix/4] (block, channel) rows."""
    npix = shard.shape[1]
    return np.ascontiguousarray(
        shard.reshape(C, RG, npix // RG).transpose(1, 0, 2).reshape(P, npix // RG)
    ).astype(BF16_NP)


_NC_CACHE = {}


def _run(feature_map, z, w1, b1, w2, b2, w3, b3, wl, bl, **spmd_kwargs):
    from concourse.bass_utils import run_bass_kernel_spmd

    feature_map = np.asarray(feature_map, dtype=np.float32)
    z = np.asarray(z, dtype=np.float32)
    w1, b1 = np.asarray(w1, np.float32), np.asarray(b1, np.float32)
    w2, b2 = np.asarray(w2, np.float32), np.asarray(b2, np.float32)
    w3, b3 = np.asarray(w3, np.float32), np.asarray(b3, np.float32)
    wl, bl = np.asarray(wl, np.float32), np.asarray(bl, np.float32)

    w13, w4s, bias = _prep_host_inputs(z, w1, b1, w2, b2, w3, b3, wl, bl)

    fm_flat = feature_map.reshape(C, VOL)
    in_maps = []
    for k in range(NCORES):
        shard = _restripe(fm_flat[:, k * NPIX:(k + 1) * NPIX])
        in_maps.append({"fm": shard, "w13": w13, "w4s": w4s, "bias": bias})

    if "nc" not in _NC_CACHE:
        _NC_CACHE["nc"] = _build_nc()
    nc = _NC_CACHE["nc"]

    res = run_bass_kernel_spmd(nc, in_maps, core_ids=list(range(NCORES)), **spmd_kwargs)
    out = np.empty((VOL,), dtype=np.float32)
    for k in range(NCORES):
        out[k * NPIX:(k + 1) * NPIX] = res.results[k]["out"]
    return out.reshape(1, 1, 96, 96, 96), res


def kernel(feature_map, z, w1, b1, w2, b2, w3, b3, wl, bl):
    out, _ = _run(feature_map, z, w1, b1, w2, b2, w3, b3, wl, bl)
    return out


# revision 30
# speedup vs baseline: 1.1313x; 1.0125x over previous
"""Trainium2 Bass kernel for nn_FComb_79319456023150 (dense_cnn).

Per-pixel MLP over a 96^3 volume: four 1x1x1 convs (38->32->32->32->1 channels
with relu between). z is batch-constant, so w1[:, 32:38] @ z folds into the
layer-1 bias and every layer becomes a K=32 channel GEMM.

Sharding: spatial (outermost X axis) across 8 cores, 110592 pixels each.
Weights/biases replicated.

Device layout per core: the host restripes each shard to [128, 27648] =
4 pixel-blocks x 32 channels on partitions, pixels on the free dim, cast to
bf16 (halves HBM traffic; fp32 PSUM accumulation keeps rel-err ~5e-3).
Layers 1-3 use a block-diagonal [128, 128] bf16 weight (4 copies of W^T on
the diagonal), so one matmul applies the 32x32 GEMM to 4 pixel blocks at
once. Work is split into 512-column slabs cycling over SEVEN single-bank
PSUM regions; the eighth bank accumulates layer 4: chunk c's sparse [128,128]
weight lands wl @ h3 on psum rows 4c+m, so one wide [rows, 512] crossing
evacuates an entire accumulation group and batched affine DMAs ship it.

Relu rides the mandatory PSUM->SBUF crossing as one whole-crossing op per
slab-layer, emitted engine-agnostic (nc.any) so the tile scheduler
load-balances ScalarE (activation Relu w/ bias) and VectorE (fused
tensor_scalar add+max). Seven regions keep ~6 crossings in flight so both
crossing engines run ~85% busy, which is the throughput bound for this
kernel; stages are emitted as a wavefront (slab s stage k at tick s+k) so
weight/input DMAs always precede their consumers in priority order.
"""

import sys

import numpy as np

if "/opt/trn_rl_repo" not in sys.path:
    sys.path.insert(0, "/opt/trn_rl_repo")

import ml_dtypes

C = 32
P = 128
RG = 4
CH = 512
VOL = 96 * 96 * 96
NCORES = 8
NPIX = VOL // NCORES       # 110592
FREE = NPIX // RG          # 27648
NCHUNKS = FREE // CH       # 54

BF16_NP = ml_dtypes.bfloat16

CFG = dict(
    l4_groups=(27, 27),
    dma_targets=(512, 1024, 2048, 3072),
    dma_steady=3584,
    slab_pattern=(512,) * 7,
    acts_bufs=7,
    wf_depth=4,
    act_share=0.505,
    attach_waits=False,
    fin_eng="dve",
)

NAME_INFO = {}


def _slabs():
    pat = CFG["slab_pattern"]
    out = []
    col, i = 0, 0
    while col < FREE:
        r = i % len(pat)
        w = min(pat[r], FREE - col)
        out.append((col, w, r))
        col += w
        i += 1
    return out


def _build_nc(npix=NPIX):
    import concourse.mybir as mybir
    from concourse import bacc
    from concourse.tile import TileContext

    cfg = CFG
    l4_groups = cfg["l4_groups"]
    assert sum(l4_groups) == NCHUNKS and max(l4_groups) * RG <= P
    # m-major psum rows (q = m*nacc + c) require uniform group sizes
    assert len(set(l4_groups)) == 1

    f32 = mybir.dt.float32
    bf16 = mybir.dt.bfloat16
    Alu = mybir.AluOpType
    Act = mybir.ActivationFunctionType

    free = npix // RG
    assert free == FREE

    nacc_max = max(l4_groups)

    nc = bacc.Bacc()
    fm = nc.dram_tensor("fm", [P, free], bf16, kind="ExternalInput")
    w13 = nc.dram_tensor("w13", [P, 3 * P], bf16, kind="ExternalInput")
    w4s = nc.dram_tensor("w4s", [P, nacc_max * P], bf16, kind="ExternalInput")
    bias = nc.dram_tensor("bias", [P, 4], f32, kind="ExternalInput")
    out = nc.dram_tensor("out", [npix], f32, kind="ExternalOutput")

    out_r = out.rearrange("(m k n) -> m k n", m=RG, k=NCHUNKS, n=CH)

    slabs = _slabs()

    # Input DMA groups: consecutive slabs batched to ramped byte targets.
    dma_groups = []
    targets = list(cfg["dma_targets"])
    cur, cw, ti = [], 0, 0
    for idx, (c0, w, r) in enumerate(slabs):
        cur.append(idx)
        cw += w
        tgt = targets[ti] if ti < len(targets) else cfg["dma_steady"]
        if cw >= tgt:
            dma_groups.append(cur)
            cur, cw = [], 0
            ti += 1
    if cur:
        dma_groups.append(cur)
    group_of_slab = {}
    for gi, g in enumerate(dma_groups):
        for idx in g:
            group_of_slab[idx] = gi

    l4_of_chunk = {}
    k = 0
    for g, n in enumerate(l4_groups):
        for c in range(n):
            l4_of_chunk[k] = (g, c, n)
            k += 1

    # crossing engine planner: stream-local alternation, seeded to equalize
    # modeled engine time (ACT is faster per op; DVE runs at 0.96 GHz)
    plan_t = {"act": 0.0, "dve": 0.0}

    def op_cost(eng, fr):
        return (fr + 222) / 1.2 if eng == "act" else (fr + 120) / 0.96

    seed_n = [0]

    def pick_seed(fr):
        pol = cfg.get("seed_policy", "greedy")
        if pol == "slab_parity":
            first = "act" if seed_n[0] % 2 == 0 else "dve"
            seed_n[0] += 1
            seq = [first, ("dve" if first == "act" else "act"), first]
            for e in seq:
                plan_t[e] += op_cost(e, fr)
            return seq
        best, arg = None, None
        for first in ("act", "dve"):
            seq = [first, ("dve" if first == "act" else "act"), first]
            t = dict(plan_t)
            for e in seq:
                t[e] += op_cost(e, fr)
            m = max(t["act"] / cfg["act_share"],
                    t["dve"] / (1 - cfg["act_share"]))
            if best is None or m < best:
                best, arg = m, seq
        for e in arg:
            plan_t[e] += op_cost(e, fr)
        return arg

    with TileContext(nc) as tc:
        with (
            tc.tile_pool(name="const", bufs=1) as constp,
            tc.tile_pool(name="data", bufs=CFG.get("data_bufs", 4)) as datap,
            tc.tile_pool(name="acts", bufs=CFG.get("acts_bufs", 4)) as actp,
            tc.tile_pool(name="outs", bufs=CFG.get("outs_bufs", 2)) as outsp,
            tc.tile_pool(name="psb", bufs=1, space="PSUM") as psb,
        ):
            w13t = constp.tile([P, 3 * P], bf16)
            btile = constp.tile([P, 4], f32)
            w4t = constp.tile([P, nacc_max * P], bf16)

            if CFG.get("preload", True):
                # dummy relu at t~0 pulls the ACT table load off the ramp
                tiny = constp.tile([1, 2], f32)
                tiny2 = constp.tile([1, 2], f32)
                nc.vector.memset(tiny[:, :], 0.0)
                pre = nc.scalar.activation(
                    tiny2[:, :], tiny[:, :], Act.Relu, bias=0.0, scale=1.0
                )
                NAME_INFO[pre.ins.name] = (-1, "preload")

            w4_half = nacc_max * P // 2
            w4_loads = [(0, w4_half), (w4_half, nacc_max * P)]

            xtiles = {}
            ob = None
            ps4 = None
            fired_const = False
            slab_h = {}
            slab_engs = {}

            def emit_stage(s, stage):
                nonlocal ob, ps4, fired_const
                c0, w, r = slabs[s]
                nch = w // CH
                if stage == 0:
                    gi = group_of_slab[s]
                    if gi not in xtiles:
                        gslabs = dma_groups[gi]
                        ga = slabs[gslabs[0]][0]
                        gb = slabs[gslabs[-1]][0] + slabs[gslabs[-1]][1]
                        xt = datap.tile([P, gb - ga], bf16, tag="x")
                        nc.sync.dma_start(xt, fm[:, ga:gb])
                        xtiles[gi] = (xt, ga)
                        if not fired_const:
                            nc.sync.dma_start(w13t, w13[:, :])
                            nc.sync.dma_start(btile, bias[:, :])
                            fired_const = True
                        elif gi in (1, 2) and w4_loads:
                            a, b = w4_loads.pop(0)
                            nc.sync.dma_start(w4t[:, a:b], w4s[:, a:b])
                    if s >= 3 and w4_loads:
                        a, b = w4_loads.pop(0)
                        nc.sync.dma_start(w4t[:, a:b], w4s[:, a:b])
                    xt, ga = xtiles[group_of_slab[s]]
                    slab_h[s] = xt[:, c0 - ga:c0 - ga + w]
                    slab_engs[s] = pick_seed(w)

                h = slab_h[s]
                if stage < 3:
                    layer = stage
                    ps = psb.tile([P, w], f32, tag=f"ps{r}")
                    wsl = w13t[:, layer * P:(layer + 1) * P]
                    for cc in range(nch):
                        mm = nc.tensor.matmul(
                            ps[:, cc * CH:(cc + 1) * CH],
                            wsl,
                            h[:, cc * CH:(cc + 1) * CH],
                            start=True,
                            stop=True,
                        )
                        NAME_INFO[mm.ins.name] = (s, f"mm{layer}.{cc}")
                    hn = actp.tile([P, w], bf16, tag=f"h{r}{layer}")
                    bcol = btile[:, layer:layer + 1]
                    # any-engine: the tile scheduler load-balances the
                    # crossing between ScalarE and VectorE. During ramp-up/
                    # ramp-down (first/last slabs) the engines are underfed,
                    # so split the crossing into two half-width ops running
                    # on both engines concurrently to shorten the serial
                    # slab chain.
                    n_slabs_t = len(slabs)
                    edge = (s < CFG.get("split_head", 0)
                            or s >= n_slabs_t - CFG.get("split_tail", 0))
                    if edge:
                        hw_ = w // 2
                        xop = nc.scalar.activation(
                            hn[:, :hw_], ps[:, :hw_], Act.Relu,
                            bias=bcol, scale=1.0,
                        )
                        xop2 = nc.vector.tensor_scalar(
                            hn[:, hw_:], ps[:, hw_:], bcol, 0.0,
                            Alu.add, Alu.max,
                        )
                        NAME_INFO[xop2.ins.name] = (s, f"relu{layer}b")
                    else:
                        xop = nc.any.tensor_scalar(
                            hn[:, :], ps[:, :], bcol, 0.0, Alu.add, Alu.max
                        )
                    NAME_INFO[xop.ins.name] = (s, f"relu{layer}")
                    slab_h[s] = hn
                    return

                # stage 3: layer 4, accumulate into the shared bank
                for cc in range(nch):
                    k = c0 // CH + cc
                    g, c, n_in_g = l4_of_chunk[k]
                    if c == 0:
                        ps4 = psb.tile([P, CH], f32, tag="psL4")
                    mm4 = nc.tensor.matmul(
                        ps4[:, :],
                        w4t[:, c * P:(c + 1) * P],
                        h[:, cc * CH:(cc + 1) * CH],
                        start=(c == 0),
                        stop=(c == n_in_g - 1),
                    )
                    NAME_INFO[mm4.ins.name] = (s, f"mm4.{cc}")
                    if c == n_in_g - 1:
                        rows = n_in_g * RG
                        k0 = k - (n_in_g - 1)
                        ob = outsp.tile([rows, CH], f32, tag="ob")
                        blcol = btile[:rows, 3:4]
                        fe = CFG.get("fin_eng", "any")
                        if fe == "act":
                            fin = nc.scalar.activation(
                                ob[:, :], ps4[:rows, :], Act.Identity,
                                bias=blcol, scale=1.0,
                            )
                        elif fe == "dve":
                            fin = nc.vector.tensor_scalar(
                                ob[:, :], ps4[:rows, :], blcol, None, Alu.add
                            )
                        else:
                            fin = nc.any.tensor_scalar(
                                ob[:, :], ps4[:rows, :], blcol, None, Alu.add
                            )
                        NAME_INFO[fin.ins.name] = (s, "final")
                        nc.sync.dma_start(
                            out_r[:, k0:k0 + n_in_g, :],
                            ob[:, :],
                        )

            # wavefront emission: slab s stage k at tick s+k, so DMA issues
            # (stage 0 of later slabs) precede older slabs' L4 reads of w4t
            n_slabs = len(slabs)
            offs = list(CFG.get("stage_offsets",
                                (0, 1, 2, CFG.get("wf_depth", 3))))
            depth = max(offs)
            stage_at = {o: i for i, o in enumerate(offs)}
            assert len(stage_at) == 4
            perm = CFG.get("tick_perm", (3, 2, 1, 0))  # stage order in tick
            for tick in range(n_slabs + depth):
                for st in perm:
                    s = tick - offs[st]
                    if 0 <= s < n_slabs:
                        emit_stage(s, st)

    import concourse.mybir as mybir_

    # Move SEQ-blocking EventSemaphore waits onto adjacent engine
    # instructions so they park in the engine wait queue instead of stalling
    # the sequencer (engine execution is in-order, so gating instruction i
    # still gates everything behind it).
    eng_ok = {}
    if CFG.get("attach_waits"):
        eng_ok = {
            mybir_.EngineType.Activation: (mybir_.InstActivation,),
            mybir_.EngineType.DVE: (mybir_.InstTensorScalarPtr,
                                    mybir_.InstMemset),
        }
    for blk in nc.main_func.blocks:
        insts = blk.instructions
        idx = 0
        while idx < len(insts):
            inst = insts[idx]
            if (isinstance(inst, mybir_.InstEventSemaphore)
                    and inst.engine in eng_ok
                    and inst.sync_info is not None
                    and len(inst.sync_info.on_wait) > 0
                    and len(inst.sync_info.on_update) == 0):
                types = eng_ok[inst.engine]
                target = None
                j = idx + 1
                while j < len(insts) and j <= idx + 12:
                    if (isinstance(insts[j], types)
                            and insts[j].engine == inst.engine):
                        target = insts[j]
                        break
                    j += 1
                if target is not None:
                    si = target.sync_info
                    if si is None:
                        target.sync_info = mybir_.SyncInfo(
                            on_wait=list(inst.sync_info.on_wait), on_update=[]
                        )
                    else:
                        si.on_wait = list(inst.sync_info.on_wait) + \
                            list(si.on_wait)
                    insts.pop(idx)
                    continue
            idx += 1

    # Walrus codegen cannot reliably attach semaphore waits to self-loading
    # matmuls; move each matmul's waits onto its preceding Ldweights
    # (engine-level wait queue) or, by default, onto a PE nop inserted just
    # before it (sequencer-side wait, same semantics).
    for blk in nc.main_func.blocks:
        insts = blk.instructions
        idx = 0
        while idx < len(insts):
            inst = insts[idx]
            if isinstance(inst, mybir_.InstMatmult):
                si = inst.sync_info
                if si is not None and len(si.on_wait) > 0:
                    target = None
                    if CFG.get("waits_on_ldw", False):
                        j = idx - 1
                        while j >= 0 and j >= idx - 3:
                            pj = insts[j]
                            if isinstance(pj, mybir_.InstLdweights) \
                                    and pj.engine == inst.engine:
                                target = pj
                                break
                            if getattr(pj, "engine", None) == inst.engine \
                                    and not isinstance(pj, mybir_.InstNoOp):
                                break
                            j -= 1
                    if target is not None:
                        tsi = target.sync_info
                        if tsi is None:
                            target.sync_info = mybir_.SyncInfo(
                                on_wait=list(si.on_wait), on_update=[]
                            )
                        else:
                            tsi.on_wait = list(tsi.on_wait) + list(si.on_wait)
                        si.on_wait = []
                    else:
                        nop = mybir_.InstNoOp(
                            name=nc.get_next_instruction_name(), ins=[], outs=[]
                        )
                        nop.engine = inst.engine
                        nop.bass_nofuse = True
                        nop.sync_info = mybir_.SyncInfo(
                            on_wait=si.on_wait, on_update=[]
                        )
                        si.on_wait = []
                        nc.register_instruction(nop)
                        insts.insert(idx, nop)
                        idx += 1
            idx += 1

    for blk in nc.main_func.blocks:
        for inst in blk.instructions:
            if isinstance(inst, mybir_.InstMatmult):
                si = inst.sync_info
                assert si is None or len(si.on_wait) == 0, inst.name

    nc.compile()
    return nc


def _blockdiag4(wT):
    o = np.zeros((P, P), dtype=np.float32)
    for b in range(RG):
        o[32 * b:32 * b + 32, 32 * b:32 * b + 32] = wT
    return o


def _prep_host_inputs(z, w1, b1, w2, b2, w3, b3, wl, bl):
    nacc = max(CFG["l4_groups"])
    f32 = np.float32
    b1e = (b1 + w1[:, C:] @ z[0]).astype(f32)

    w13 = np.concatenate(
        [_blockdiag4(w1[:, :C].T), _blockdiag4(w2.T), _blockdiag4(w3.T)],
        axis=1,
    ).astype(BF16_NP)

    w4s = np.zeros((P, nacc * P), dtype=f32)
    for c in range(nacc):
        for m in range(RG):
            w4s[32 * m:32 * m + 32, c * P + m * nacc + c] = wl[0, :]
    w4s = w4s.astype(BF16_NP)

    bias = np.zeros((P, 4), dtype=f32)
    bias[:, 0] = np.tile(b1e, RG)
    bias[:, 1] = np.tile(b2.astype(f32), RG)
    bias[:, 2] = np.tile(b3.astype(f32), RG)
    bias[:, 3] = f32(bl[0])
    return w13, w4s, bias


def _restripe(shard):
    """[32, npix] channel-major shard -> [128, npix/4] (block, channel) rows."""
    npix = shard.shape[1]
    return np.ascontiguousarray(
        shard.reshape(C, RG, npix // RG).transpose(1, 0, 2).reshape(P, npix // RG)
    ).astype(BF16_NP)


_NC_CACHE = {}


def _run(feature_map, z, w1, b1, w2, b2, w3, b3, wl, bl, **spmd_kwargs):
    from concourse.bass_utils import run_bass_kernel_spmd

    feature_map = np.asarray(feature_map, dtype=np.float32)
    z = np.asarray(z, dtype=np.float32)
    w1, b1 = np.asarray(w1, np.float32), np.asarray(b1, np.float32)
    w2, b2 = np.asarray(w2, np.float32), np.asarray(b2, np.float32)
    w3, b3 = np.asarray(w3, np.float32), np.asarray(b3, np.float32)
    wl, bl = np.asarray(wl, np.float32), np.asarray(bl, np.float32)

    w13, w4s, bias = _prep_host_inputs(z, w1, b1, w2, b2, w3, b3, wl, bl)

    fm_flat = feature_map.reshape(C, VOL)
    in_maps = []
    for k in range(NCORES):
        shard = _restripe(fm_flat[:, k * NPIX:(k + 1) * NPIX])
        in_maps.append({"fm": shard, "w13": w13, "w4s": w4s, "bias": bias})

    if "nc" not in _NC_CACHE:
        _NC_CACHE["nc"] = _build_nc()
    nc = _NC_CACHE["nc"]

    res = run_bass_kernel_spmd(nc, in_maps, core_ids=list(range(NCORES)), **spmd_kwargs)
    out = np.empty((VOL,), dtype=np.float32)
    for k in range(NCORES):
        out[k * NPIX:(k + 1) * NPIX] = res.results[k]["out"]
    return out.reshape(1, 1, 96, 96, 96), res


def kernel(feature_map, z, w1, b1, w2, b2, w3, b3, wl, bl):
    out, _ = _run(feature_map, z, w1, b1, w2, b2, w3, b3, wl, bl)
    return out
